# revision 20
# baseline (speedup 1.0000x reference)
"""Causal single-head attention kernel for Trainium2 (8 NeuronCores).

Problem: context [8, 2048, 1024] fp32, Wq/Wk/Wv [1024, 64] fp32.
  q/k/v = context @ W; scores = q k^T / sqrt(64) causal-masked; softmax; out = wei @ v.

Sharding: data-parallel over batch — one batch element per core (B == n_cores == 8).

Per-core dataflow (all "transposed" to avoid large on-chip transposes):
  - ctx fp32 is cast to bf16 in-flight by the DMA (SWDGE cast), then each
    [128,128] tile is transposed on the PE into ctxT [d, T] layout.
  - Projections contract over d: one matmul packs Q+V (M=128 stationary
    [Wq|Wv]), K runs separately (M=64).  Outputs are qT/kT [64, T] (h on
    partitions) and vT on partitions 64..127 (its consumer is a PE
    transpose, which can read there via the identity's diagonal block).
  - scoresT[j, i] = kT.T @ qT per 128-row j-block, 512-col i-range, fp32 in
    PSUM.  Causal masking adds -1e30 on the diagonal block via an
    identity-matmul accumulate; no max-subtraction (scores are O(7), exp
    fits fp32 comfortably).
  - exp on ACT (scale=1/8) PSUM->SBUF bf16.
  - out^T accumulation: lhsT = v_aug [j, 65] (col 64 = ones so row 64 of
    out^T is the softmax denominator), rhs = exp scores.
  - out^T [65, 512] -> PE transpose -> [128, 65]; DVE reciprocal +
    tensor_scalar_mul normalizes; DMA fp32 rows back to HBM.
"""

import numpy as np
from contextlib import ExitStack

import concourse.bass as bass
import concourse.mybir as mybir
import concourse.tile as tile
from concourse import bacc
from concourse.bass_utils import run_bass_kernel_spmd
from concourse.masks import make_identity

F32 = mybir.dt.float32
BF16 = mybir.dt.bfloat16

B = 8
FULL_T, FULL_D, FULL_HS = 2048, 1024, 64
NEG = -1e30


def build_attention_nc(T=FULL_T, D=FULL_D, HS=FULL_HS, repeat=1):
    """Build the per-core Bass program. T must be a multiple of 512.

    repeat > 1 wraps the body in an on-device For_i loop (timing ruler only).
    """
    assert T % 512 == 0 and D % 128 == 0 and HS == 64
    DC = D // 128          # d-chunks (contraction tiles)
    NR = T // 512          # i-ranges
    NJ = T // 128          # j-blocks
    TR = 512               # i-range width

    nc = bacc.Bacc("TRN2", target_bir_lowering=False, debug=False)
    ctx_d = nc.dram_tensor("context", [T, D], F32, kind="ExternalInput").ap()
    wq_d = nc.dram_tensor("Wq", [D, HS], F32, kind="ExternalInput").ap()
    wk_d = nc.dram_tensor("Wk", [D, HS], F32, kind="ExternalInput").ap()
    wv_d = nc.dram_tensor("Wv", [D, HS], F32, kind="ExternalInput").ap()
    out_d = nc.dram_tensor("out", [T, HS], F32, kind="ExternalOutput").ap()

    with tile.TileContext(nc) as tc, ExitStack() as ctx:
        const = ctx.enter_context(tc.tile_pool(name="const", bufs=1))
        cbf_pool = ctx.enter_context(tc.tile_pool(name="cbf", bufs=6))
        big = ctx.enter_context(tc.tile_pool(name="big", bufs=1))
        exp_pool = ctx.enter_context(tc.tile_pool(name="expp", bufs=4))
        osb_pool = ctx.enter_context(tc.tile_pool(name="osb", bufs=2))
        small = ctx.enter_context(tc.tile_pool(name="small", bufs=4))
        tp_ps = ctx.enter_context(
            tc.tile_pool(name="tp_ps", bufs=1, space=bass.MemorySpace.PSUM))
        pj_ps = ctx.enter_context(
            tc.tile_pool(name="pj_ps", bufs=1, space=bass.MemorySpace.PSUM))
        sc_ps = ctx.enter_context(
            tc.tile_pool(name="sc_ps", bufs=3, space=bass.MemorySpace.PSUM))
        ot_ps = ctx.enter_context(
            tc.tile_pool(name="ot_ps", bufs=2, space=bass.MemorySpace.PSUM))

        # ---- kick off ctx loads first so PE isn't blocked on the const
        # setup queue (gpsimd) at startup ----
        cbf_tiles = {}
        if repeat == 1:
            for r in range(NR):
                cbf = cbf_pool.tile([128, 4, D], BF16, tag="cbf")
                nc.gpsimd.dma_start(
                    cbf[:],
                    ctx_d[r * TR:(r + 1) * TR, :].rearrange("(a p) d -> p a d", p=128))
                cbf_tiles[r] = cbf

        # ---- constants ----
        ident_bf = const.tile([128, 128], BF16)
        make_identity(nc, ident_bf[:])
        ident_f = const.tile([128, 128], F32)
        make_identity(nc, ident_f[:])
        # tri[j, u] = 1 if j <= u else 0 — multiplies the diagonal block of
        # the exp'd scores to zero the masked (j > i) entries.
        tri = const.tile([128, 128], BF16)
        nc.gpsimd.memset(tri[:], 1.0)
        nc.gpsimd.affine_select(
            out=tri[:], in_=tri[:],
            compare_op=mybir.AluOpType.is_ge, fill=0.0,
            base=0, pattern=[[1, 128]], channel_multiplier=-1)

        # weights: Wqv packs [Wq | Wv] (cols 0:64 / 64:128); Wk separate.
        wqv = const.tile([128, DC, 128], BF16)
        nc.gpsimd.dma_start(wqv[:, :, 0:64], wq_d.rearrange("(c p) h -> p c h", p=128))
        nc.gpsimd.dma_start(wqv[:, :, 64:128], wv_d.rearrange("(c p) h -> p c h", p=128))
        wk = const.tile([128, DC, 64], BF16)
        nc.gpsimd.dma_start(wk[:], wk_d.rearrange("(c p) h -> p c h", p=128))

        # ---- big persistent tiles ----
        ctxT = big.tile([128, DC, T], BF16)       # [d_in_chunk, chunk, t]
        S = big.tile([64, 2, T], BF16)            # [h, {q=0, k=1}, t]
        vhi = big.tile([128, T], BF16)            # rows 64:128 hold vT
        vaug = big.tile([128, NJ, 80], BF16)      # [j, jb, h]; col 64 = ones
        nc.gpsimd.memset(vaug[:, :, 64:65], 1.0)

        def body(_iv=None):
          for r in range(NR):
            i0 = r * TR
            # ---- load ctx rows [i0, i0+512) (one cast-DMA) and transpose ----
            if r in cbf_tiles:
                cbf = cbf_tiles[r]
            else:
                cbf = cbf_pool.tile([128, 4, D], BF16, tag="cbf")
                nc.gpsimd.dma_start(
                    cbf[:], ctx_d[i0:i0 + TR, :].rearrange("(a p) d -> p a d", p=128))
            for c in range(DC):
                tp = tp_ps.tile([128, 512], BF16, tag="tp")
                for tt in range(4):
                    nc.tensor.transpose(
                        tp[:, tt * 128:(tt + 1) * 128],
                        cbf[:, tt, c * 128:(c + 1) * 128],
                        ident_bf[:])
                nc.vector.tensor_copy(ctxT[:, c, i0:i0 + TR], tp[:])

            # ---- projections for this range ----
            pj = pj_ps.tile([128, 1024], F32, tag="pj")
            for c in range(DC):
                nc.tensor.matmul(pj[:, 0:512], wqv[:, c, :], ctxT[:, c, i0:i0 + TR],
                                 start=(c == 0), stop=(c == DC - 1))
                nc.tensor.matmul(pj[0:64, 512:1024], wk[:, c, :], ctxT[:, c, i0:i0 + TR],
                                 start=(c == 0), stop=(c == DC - 1))
            nc.vector.tensor_copy(S[:, 0, i0:i0 + TR], pj[0:64, 0:512])       # qT
            nc.scalar.copy(S[:, 1, i0:i0 + TR], pj[0:64, 512:1024])    # kT
            nc.vector.tensor_copy(vhi[64:128, i0:i0 + TR], pj[64:128, 0:512])  # vT

            # ---- v_aug for the 4 new j-blocks ----
            for jb in range(4 * r, 4 * r + 4):
                vtp = tp_ps.tile([128, 64], BF16, tag="tp")
                nc.tensor.transpose(
                    vtp[:], vhi[64:128, jb * 128:(jb + 1) * 128],
                    ident_bf[64:128, 64:128])
                nc.vector.tensor_copy(vaug[:, jb, 0:64], vtp[:])

            # ---- attention over j-blocks (pairs share a 2-bank psum tile) ----
            oT = ot_ps.tile([65, 512], F32, tag="ot")
            n_jb = 4 * r + 4
            for jb in range(n_jb):
                o = 0 if jb < 4 * r else (jb - 4 * r) * 128
                sc = sc_ps.tile([128, 512], F32, tag="sc")
                exp_sb = exp_pool.tile([128, 512], BF16, tag="exp")
                nc.tensor.matmul(
                    sc[:, o:512],
                    S[:, 1, jb * 128:(jb + 1) * 128],
                    S[:, 0, i0 + o:i0 + TR],
                    start=True, stop=True)
                nc.scalar.activation(
                    exp_sb[:, o:512], sc[:, o:512],
                    mybir.ActivationFunctionType.Exp,
                    scale=float(HS) ** -0.5)
                if jb >= 4 * r:
                    # zero the masked upper part of the diagonal block
                    nc.vector.tensor_mul(
                        exp_sb[:, o:o + 128], exp_sb[:, o:o + 128], tri[:])
                nc.tensor.matmul(
                    oT[:, o:512],
                    vaug[:, jb, 0:65],
                    exp_sb[:, o:512],
                    start=(jb == 0), stop=(jb == n_jb - 1))

            # ---- normalize + output ----
            # Transpose s maps columns s, s+4, s+8, ... so partition p ends up
            # holding rows 4p..4p+3 of the range — the DMA then writes 1 KiB
            # contiguous per partition (128 descriptors instead of 512).
            oT_sb = osb_pool.tile([65, 512], F32, tag="otsb")
            nc.scalar.copy(oT_sb[:], oT[:])
            out_sb = osb_pool.tile([128, 4, 64], F32, tag="outsb")
            on_ps = ot_ps.tile([128, 4, 65], F32, tag="ot")
            for s in range(4):
                nc.tensor.transpose(
                    on_ps[:, s, :], oT_sb[:, s::4], ident_f[0:65, 0:65])
                rec = small.tile([128, 1], F32, tag="rec")
                nc.vector.reciprocal(rec[:], on_ps[:, s, 64:65])
                nc.vector.tensor_scalar_mul(out_sb[:, s, :], on_ps[:, s, 0:64], rec[:])
            nc.sync.dma_start(
                out_d[i0:i0 + TR, :].rearrange("(p a) h -> p a h", p=128),
                out_sb[:])

        if repeat > 1:
            tc.For_i_unrolled(0, repeat, 1, body, max_unroll=1)
        else:
            body()

    nc.compile()
    return nc


_NC_CACHE = {}


def _get_nc(T=FULL_T, D=FULL_D, HS=FULL_HS):
    key = (T, D, HS)
    if key not in _NC_CACHE:
        _NC_CACHE[key] = build_attention_nc(T, D, HS)
    return _NC_CACHE[key]


def kernel(context, Wq, Wk, Wv):
    context = np.ascontiguousarray(np.asarray(context, dtype=np.float32))
    Wq = np.ascontiguousarray(np.asarray(Wq, dtype=np.float32))
    Wk = np.ascontiguousarray(np.asarray(Wk, dtype=np.float32))
    Wv = np.ascontiguousarray(np.asarray(Wv, dtype=np.float32))
    b, t, d = context.shape
    nc = _get_nc(T=t, D=d, HS=Wq.shape[1])
    in_maps = [
        {"context": context[i], "Wq": Wq, "Wk": Wk, "Wv": Wv} for i in range(b)
    ]
    res = run_bass_kernel_spmd(nc, in_maps, core_ids=list(range(b)))
    return np.stack([r["out"] for r in res.results], axis=0)


if __name__ == "__main__":
    rng = np.random.default_rng(0)
    ctx = rng.standard_normal((B, FULL_T, FULL_D), dtype=np.float32)
    ws = [rng.standard_normal((FULL_D, FULL_HS), dtype=np.float32) * FULL_D ** -0.5
          for _ in range(3)]
    out = kernel(ctx, *ws)
    print(out.shape, out.dtype)


# revision 22
# speedup vs baseline: 1.2601x; 1.2601x over previous
"""Causal single-head attention kernel for Trainium2 (8 NeuronCores).

Problem: context [8, 2048, 1024] fp32, Wq/Wk/Wv [1024, 64] fp32.
  q/k/v = context @ W; scores = q k^T / sqrt(64) causal-masked; softmax; out = wei @ v.

Sharding: data-parallel over batch — one batch element per core (B == n_cores == 8).

Per-core dataflow (all "transposed" to avoid large on-chip transposes):
  - ctx fp32 is cast to bf16 in-flight by the DMA (SWDGE cast), then each
    [128,128] tile is transposed on the PE into ctxT [d, T] layout.
  - Projections contract over d: one matmul packs Q+V (M=128 stationary
    [Wq|Wv]), K runs separately (M=64).  Outputs are qT/kT [64, T] (h on
    partitions) and vT on partitions 64..127 (its consumer is a PE
    transpose, which can read there via the identity's diagonal block).
  - scoresT[j, i] = kT.T @ qT per 128-row j-block, 512-col i-range, fp32 in
    PSUM.  Causal masking adds -1e30 on the diagonal block via an
    identity-matmul accumulate; no max-subtraction (scores are O(7), exp
    fits fp32 comfortably).
  - exp on ACT (scale=1/8) PSUM->SBUF bf16.
  - out^T accumulation: lhsT = v_aug [j, 65] (col 64 = ones so row 64 of
    out^T is the softmax denominator), rhs = exp scores.
  - out^T [65, 512] -> PE transpose -> [128, 65]; DVE reciprocal +
    tensor_scalar_mul normalizes; DMA fp32 rows back to HBM.
"""

import numpy as np
from contextlib import ExitStack

import concourse.bass as bass
import concourse.mybir as mybir
import concourse.tile as tile
from concourse import bacc
from concourse.bass_utils import run_bass_kernel_spmd
from concourse.masks import make_identity

F32 = mybir.dt.float32
BF16 = mybir.dt.bfloat16

B = 8
FULL_T, FULL_D, FULL_HS = 2048, 1024, 64
NEG = -1e30


def build_attention_nc(T=FULL_T, D=FULL_D, HS=FULL_HS, repeat=1):
    """Build the per-core Bass program. T must be a multiple of 512.

    repeat > 1 wraps the body in an on-device For_i loop (timing ruler only).
    """
    assert T % 512 == 0 and D % 128 == 0 and HS == 64
    DC = D // 128          # d-chunks (contraction tiles)
    NR = T // 512          # i-ranges
    NJ = T // 128          # j-blocks
    TR = 512               # i-range width

    nc = bacc.Bacc("TRN2", target_bir_lowering=False, debug=False)
    ctx_d = nc.dram_tensor("context", [T, D], F32, kind="ExternalInput").ap()
    wq_d = nc.dram_tensor("Wq", [D, HS], F32, kind="ExternalInput").ap()
    wk_d = nc.dram_tensor("Wk", [D, HS], F32, kind="ExternalInput").ap()
    wv_d = nc.dram_tensor("Wv", [D, HS], F32, kind="ExternalInput").ap()
    out_d = nc.dram_tensor("out", [T, HS], F32, kind="ExternalOutput").ap()

    with tile.TileContext(nc) as tc, ExitStack() as ctx:
        const = ctx.enter_context(tc.tile_pool(name="const", bufs=1))
        cbf_pool = ctx.enter_context(tc.tile_pool(name="cbf", bufs=6))
        big = ctx.enter_context(tc.tile_pool(name="big", bufs=1))
        exp_pool = ctx.enter_context(tc.tile_pool(name="expp", bufs=4))
        osb_pool = ctx.enter_context(tc.tile_pool(name="osb", bufs=2))
        small = ctx.enter_context(tc.tile_pool(name="small", bufs=4))
        tp_ps = ctx.enter_context(
            tc.tile_pool(name="tp_ps", bufs=2, space=bass.MemorySpace.PSUM))
        pj_ps = ctx.enter_context(
            tc.tile_pool(name="pj_ps", bufs=1, space=bass.MemorySpace.PSUM))
        sc_ps = ctx.enter_context(
            tc.tile_pool(name="sc_ps", bufs=2, space=bass.MemorySpace.PSUM))
        ot_ps = ctx.enter_context(
            tc.tile_pool(name="ot_ps", bufs=2, space=bass.MemorySpace.PSUM))

        # ---- kick off ctx loads first so PE isn't blocked on the const
        # setup queue (gpsimd) at startup ----
        cbf_tiles = {}
        if repeat == 1:
            for r in range(NR):
                cbf = cbf_pool.tile([128, 4, D], BF16, tag="cbf")
                nc.gpsimd.dma_start(
                    cbf[:],
                    ctx_d[r * TR:(r + 1) * TR, :].rearrange("(a p) d -> p a d", p=128))
                cbf_tiles[r] = cbf

        # ---- constants ----
        ident_bf = const.tile([128, 128], BF16)
        make_identity(nc, ident_bf[:])
        ident_f = const.tile([128, 128], F32)
        make_identity(nc, ident_f[:])
        # tri[j, u] = 1 if j <= u else 0 — multiplies the diagonal block of
        # the exp'd scores to zero the masked (j > i) entries.
        tri = const.tile([128, 128], BF16)
        nc.gpsimd.memset(tri[:], 1.0)
        nc.gpsimd.affine_select(
            out=tri[:], in_=tri[:],
            compare_op=mybir.AluOpType.is_ge, fill=0.0,
            base=0, pattern=[[1, 128]], channel_multiplier=-1)

        # weights: Wqv packs [Wq | Wv] (cols 0:64 / 64:128); Wk separate.
        wqv = const.tile([128, DC, 128], BF16)
        nc.gpsimd.dma_start(wqv[:, :, 0:64], wq_d.rearrange("(c p) h -> p c h", p=128))
        nc.gpsimd.dma_start(wqv[:, :, 64:128], wv_d.rearrange("(c p) h -> p c h", p=128))
        wk = const.tile([128, DC, 64], BF16)
        nc.gpsimd.dma_start(wk[:], wk_d.rearrange("(c p) h -> p c h", p=128))

        # ---- big persistent tiles ----
        ctxT = big.tile([128, DC, T], BF16)       # [d_in_chunk, chunk, t]
        S = big.tile([64, 2, T], BF16)            # [h, {q=0, k=1}, t]
        vhi = big.tile([128, T], BF16)            # rows 64:128 hold vT
        vaug = big.tile([128, NJ, 80], BF16)      # [j, jb, h]; col 64 = ones
        nc.gpsimd.memset(vaug[:, :, 64:65], 1.0)

        def body(_iv=None):
          for r in range(NR):
            i0 = r * TR
            # ---- load ctx rows [i0, i0+512) (one cast-DMA) and transpose ----
            if r in cbf_tiles:
                cbf = cbf_tiles[r]
            else:
                cbf = cbf_pool.tile([128, 4, D], BF16, tag="cbf")
                nc.gpsimd.dma_start(
                    cbf[:], ctx_d[i0:i0 + TR, :].rearrange("(a p) d -> p a d", p=128))
            for c in range(DC):
                tp = tp_ps.tile([128, 512], BF16, tag="tp")
                for tt in range(4):
                    nc.tensor.transpose(
                        tp[:, tt * 128:(tt + 1) * 128],
                        cbf[:, tt, c * 128:(c + 1) * 128],
                        ident_bf[:])
                nc.vector.tensor_copy(ctxT[:, c, i0:i0 + TR], tp[:])

            # ---- projections for this range ----
            pj = pj_ps.tile([128, 1024], F32, tag="pj")
            for c in range(DC):
                nc.tensor.matmul(pj[:, 0:512], wqv[:, c, :], ctxT[:, c, i0:i0 + TR],
                                 start=(c == 0), stop=(c == DC - 1))
                nc.tensor.matmul(pj[0:64, 512:1024], wk[:, c, :], ctxT[:, c, i0:i0 + TR],
                                 start=(c == 0), stop=(c == DC - 1))
            nc.vector.tensor_copy(S[:, 0, i0:i0 + TR], pj[0:64, 0:512])       # qT
            nc.scalar.copy(S[:, 1, i0:i0 + TR], pj[0:64, 512:1024])    # kT
            nc.vector.tensor_copy(vhi[64:128, i0:i0 + TR], pj[64:128, 0:512])  # vT

            # ---- v_aug for the 4 new j-blocks ----
            for jb in range(4 * r, 4 * r + 4):
                vtp = tp_ps.tile([128, 64], BF16, tag="tp")
                nc.tensor.transpose(
                    vtp[:], vhi[64:128, jb * 128:(jb + 1) * 128],
                    ident_bf[64:128, 64:128])
                nc.vector.tensor_copy(vaug[:, jb, 0:64], vtp[:])

            # ---- attention over j-blocks (pairs share a 2-bank psum tile) ----
            oT = ot_ps.tile([65, 512], F32, tag="ot")
            n_jb = 4 * r + 4
            for jb in range(n_jb):
                o = 0 if jb < 4 * r else (jb - 4 * r) * 128
                sc = sc_ps.tile([128, 512], F32, tag="sc")
                exp_sb = exp_pool.tile([128, 512], BF16, tag="exp")
                nc.tensor.matmul(
                    sc[:, o:512],
                    S[:, 1, jb * 128:(jb + 1) * 128],
                    S[:, 0, i0 + o:i0 + TR],
                    start=True, stop=True)
                nc.scalar.activation(
                    exp_sb[:, o:512], sc[:, o:512],
                    mybir.ActivationFunctionType.Exp,
                    scale=float(HS) ** -0.5)
                if jb >= 4 * r:
                    # zero the masked upper part of the diagonal block
                    nc.vector.tensor_mul(
                        exp_sb[:, o:o + 128], exp_sb[:, o:o + 128], tri[:])
                nc.tensor.matmul(
                    oT[:, o:512],
                    vaug[:, jb, 0:65],
                    exp_sb[:, o:512],
                    start=(jb == 0), stop=(jb == n_jb - 1))

            # ---- normalize + output ----
            # Transpose s maps columns s, s+4, s+8, ... so partition p ends up
            # holding rows 4p..4p+3 of the range — the DMA then writes 1 KiB
            # contiguous per partition (128 descriptors instead of 512).
            oT_sb = osb_pool.tile([65, 512], F32, tag="otsb")
            nc.scalar.copy(oT_sb[:], oT[:])
            out_sb = osb_pool.tile([128, 4, 64], F32, tag="outsb")
            on_ps = ot_ps.tile([128, 4, 65], F32, tag="ot")
            for s in range(4):
                nc.tensor.transpose(
                    on_ps[:, s, :], oT_sb[:, s::4], ident_f[0:65, 0:65])
                rec = small.tile([128, 1], F32, tag="rec")
                nc.vector.reciprocal(rec[:], on_ps[:, s, 64:65])
                nc.vector.tensor_scalar_mul(out_sb[:, s, :], on_ps[:, s, 0:64], rec[:])
            nc.sync.dma_start(
                out_d[i0:i0 + TR, :].rearrange("(p a) h -> p a h", p=128),
                out_sb[:])

        if repeat > 1:
            tc.For_i_unrolled(0, repeat, 1, body, max_unroll=1)
        else:
            body()

    nc.compile()
    return nc


_NC_CACHE = {}


def _get_nc(T=FULL_T, D=FULL_D, HS=FULL_HS):
    key = (T, D, HS)
    if key not in _NC_CACHE:
        _NC_CACHE[key] = build_attention_nc(T, D, HS)
    return _NC_CACHE[key]


def kernel(context, Wq, Wk, Wv):
    context = np.ascontiguousarray(np.asarray(context, dtype=np.float32))
    Wq = np.ascontiguousarray(np.asarray(Wq, dtype=np.float32))
    Wk = np.ascontiguousarray(np.asarray(Wk, dtype=np.float32))
    Wv = np.ascontiguousarray(np.asarray(Wv, dtype=np.float32))
    b, t, d = context.shape
    nc = _get_nc(T=t, D=d, HS=Wq.shape[1])
    in_maps = [
        {"context": context[i], "Wq": Wq, "Wk": Wk, "Wv": Wv} for i in range(b)
    ]
    res = run_bass_kernel_spmd(nc, in_maps, core_ids=list(range(b)))
    return np.stack([r["out"] for r in res.results], axis=0)


if __name__ == "__main__":
    rng = np.random.default_rng(0)
    ctx = rng.standard_normal((B, FULL_T, FULL_D), dtype=np.float32)
    ws = [rng.standard_normal((FULL_D, FULL_HS), dtype=np.float32) * FULL_D ** -0.5
          for _ in range(3)]
    out = kernel(ctx, *ws)
    print(out.shape, out.dtype)
